# revision 1
# baseline (speedup 1.0000x reference)
"""Trainium2 Bass kernel for the AgentEncoder GNN (nn_AgentEncoder_75840532512965).

Self-contained: kernel(**inputs) takes the FULL unsharded inputs (as in
reference.setup_inputs()), shards edges by target across 8 NeuronCores,
runs a Bass/Tile SPMD kernel via concourse.bass_utils.run_bass_kernel_spmd,
and returns the full [100000, 128] float32 output.

See the build() docstring for the kernel design.
"""
import concourse.tile as tile
from concourse.vector_clock import ScopedClock, VectorClock

"""Workaround for this walrus build: Drain/NoOp instructions (NO_STRUCT
codegen path) accept at most 2 sync waits. Tile's tail drain attaches one
wait per active logical processor, which can exceed that. Split the waits
one-per-NOP before the drain so the drain itself needs none."""

_orig_drain_and_barrier = tile.TileContext._drain_and_barrier


def _patched_drain_and_barrier(self, tick_clock, wait_clock):
    nc = self.nc
    gc = tick_clock.global_clock
    vec = list(gc)
    n = len(vec)
    for i, v in enumerate(vec):
        if v <= 0:
            continue
        partial = [0] * n
        partial[i] = v
        nopi = nc.sync.nop(nofuse=True, hint=f"tailwait{i}")
        wait_clock.add_sem_waits(
            nopi.ins, ScopedClock({None: VectorClock(partial)})
        )
    # replicate _orig_drain_and_barrier but with no waits on the drain
    # (SP executes it after the wait-nops above, same engine, in order)
    nc.sync.drain()
    nc.all_engine_barrier()
    popped = nc._tile_sem_poison_stack.pop()
    assert popped is self._sem_poison
    nc.clear_and_free_semaphores(list(self.sems.allocated().values()))
    nc.all_engine_barrier()


MAX_WAITS = 1


def _split_waits_json(raw: bytes) -> bytes:
    import json

    m = json.loads(raw)
    changed = False
    for f in m.get("functions", []):
        for bb in f.get("blocks", []):
            insts = bb.get("instructions", [])
            out = []
            for inst in insts:
                si = inst.get("sync_info")
                ow = (si or {}).get("on_wait") or []
                if len(ow) > MAX_WAITS:
                    extra = ow[:-MAX_WAITS]
                    keep = ow[-MAX_WAITS:]
                    si["on_wait"] = keep
                    for i in range(0, len(extra), MAX_WAITS):
                        out.append(
                            {
                                "debug": inst.get("debug", 0),
                                "engine": inst["engine"],
                                "ins": [],
                                "name": f"{inst['name']}_w{i}",
                                "opcode": "NoOp",
                                "outs": [],
                                "sync_info": {
                                    "on_update": [],
                                    "on_wait": extra[i : i + MAX_WAITS],
                                },
                                "text_hint": "waitsplit",
                            }
                        )
                    changed = True
                out.append(inst)
            bb["instructions"] = out
    if not changed:
        return raw
    return json.dumps(m).encode()


def _install_patches():
    import concourse.bass as bass

    tile.TileContext._drain_and_barrier = _patched_drain_and_barrier
    if not getattr(bass.Bass, "_waitsplit_patched", False):
        orig_to_json = bass.Bass.to_json_bytes

        def to_json_bytes(self, *a, **k):
            return _split_waits_json(orig_to_json(self, *a, **k))

        bass.Bass.to_json_bytes = to_json_bytes
        bass.Bass._waitsplit_patched = True


"""GNN message-passing kernel (AgentEncoder) for 8 TRN2 NeuronCores. v2.

Design:
- Nodes sharded by contiguous 128-node tiles: core c owns nodes
  [c*n_local, (c+1)*n_local), n_local = T*128. Edges sharded by TARGET.
- Within a core, edges sorted by target tile, padded per tile to K = Kc*128.
- Per tile+type: ONE batched indirect gather of all K source rows (bf16);
  one-hot selector matrices built on-device: A [e,n] via a single batched
  is_equal (3D broadcast APs), Msel [n,e] via ones-matmul broadcast of the
  rel-row + is_equal vs a partition-iota. Per 128-edge chunk: PE transpose
  of gathered rows, 3 accumulating matmuls for pre1, relu, scatter matmul.
- W2 applied per-node after aggregation (linearity), batched 4 tiles wide
  in feature-major layout; LN + comb-MLP fused there too.
- Layer 0 -> 1 h exchange: single AllGather (bf16).
- SPMD trick: layer-0 gather table is per-core ROTATED (own nodes first) so
  "own slice" addressing is uniform; indices are rotated on host. Layer-1
  table (AllGather output) is in global layout.
"""
import numpy as np
import ml_dtypes

import concourse.bass as bass
import concourse.mybir as mybir
import concourse.tile as tile
from concourse.masks import make_identity

_install_patches()

DT = mybir.dt
BF = DT.bfloat16
F32 = DT.float32
I32 = DT.int32
AF = mybir.ActivationFunctionType
OP = mybir.AluOpType

H = 128
F_IN = 39
L = 2
EPS = 1e-5
TYPES = ("ally", "enc")


class Cfg:
    def __init__(self, n_cores=8, T=98, Kc=7):
        self.n_cores = n_cores
        self.T = T            # 128-node tiles per core
        self.Kc = Kc          # 128-edge chunks per tile (padded)
        self.K = Kc * 128
        self.n_local = T * 128
        self.NPAD = n_cores * self.n_local
        self.groups = []
        k = 0
        while k < T:
            nt = min(4, T - k)
            self.groups.append((k, nt))
            k += nt
        # AllGather chunks: split groups into ~4 runs; per chunk (g_lo, g_hi, row_base, rows)
        ng = len(self.groups)
        if ng >= 8:
            # decreasing chunk sizes so the LAST AllGather (the exposed one)
            # is small; fractions of groups: 32%, 28%, 24%, 16%
            fr = [0.0, 0.36, 0.68, 0.92, 1.0]
            bounds = [round(f * ng) for f in fr]
        else:
            bounds = [round(i * ng / 4) for i in range(5)]
        self.cc_chunks = []
        for i in range(4):
            glo, ghi = bounds[i], bounds[i + 1]
            if glo == ghi:
                continue
            r0 = self.groups[glo][0] * 128
            r1 = (self.groups[ghi - 1][0] + self.groups[ghi - 1][1]) * 128
            self.cc_chunks.append((glo, ghi, r0, r1 - r0))


def host_prep(cfg, inputs):
    ncores, T, Kc, K = cfg.n_cores, cfg.T, cfg.Kc, cfg.K
    NL, NPAD = cfg.n_local, cfg.NPAD
    N = inputs["x"].shape[0]
    G = ncores * T

    x = np.asarray(inputs["x"], np.float32)
    xT_pad = np.zeros((F_IN, NPAD), np.float32)
    xT_pad[:, :N] = x.T

    pertype = {}
    for t in TYPES:
        ei = np.asarray(inputs[f"{t}_ei"])
        ea = np.asarray(inputs[f"{t}_ea"], np.float32)
        src, tgt = ei[0].astype(np.int64), ei[1].astype(np.int64)
        order = np.argsort(tgt, kind="stable")
        s_src, s_tgt, s_ea = src[order], tgt[order], ea[order]
        tile_id = s_tgt // 128
        counts = np.bincount(tile_id, minlength=G)
        need = int(np.ceil(counts.max() / 128))
        assert need <= Kc, f"Kc={Kc} too small, need {need}"
        offs = np.zeros(G + 1, np.int64)
        offs[1:] = np.cumsum(counts)
        col = np.arange(len(s_tgt)) - offs[tile_id]
        src_p = np.zeros((G, K), np.int64)
        rel_p = np.full((G, K), -1, np.int64)
        ea_p = np.zeros((G, 4, K), np.float32)
        src_p[tile_id, col] = s_src
        rel_p[tile_id, col] = s_tgt - tile_id * 128
        ea_p[tile_id, 0, col] = s_ea[:, 0]
        ea_p[tile_id, 1, col] = s_ea[:, 1]
        ea_p[tile_id, 2, col] = 1.0
        ea_p[:, 3, :] = rel_p  # rel row for Msel bcast (bf16-exact small ints)
        deg = np.bincount(tgt, minlength=NPAD).astype(np.float32)
        pertype[t] = (src_p, rel_p, ea_p, deg)

    # layer-1 table layout: chunk-major [sum_c 8*rows_c], position =
    # 8*base_c + rank*rows_c + (local - base_c)
    cbases = np.array([cc[2] for cc in cfg.cc_chunks], dtype=np.int64)
    crows = np.array([cc[3] for cc in cfg.cc_chunks], dtype=np.int64)
    gbase = np.concatenate([[0], np.cumsum(crows * ncores)])[:-1]

    def remap_l1(g):
        r = g // NL
        x = g % NL
        ci = np.searchsorted(cbases, x, side="right") - 1
        return gbase[ci] + r * crows[ci] + (x - cbases[ci])

    def edge_layout(arr_gk, c):
        # [T, K] for core c -> [128, T*Kc]: edge j of tile k -> (j%128, k*Kc + j//128)
        a = arr_gk[c * T : (c + 1) * T].reshape(T, Kc, 128)
        return np.ascontiguousarray(a.transpose(2, 0, 1).reshape(128, T * Kc))

    def interleave_ir(idx_l, rel_l):
        a = idx_l.reshape(128, T, Kc)
        b = rel_l.reshape(128, T, Kc)
        return np.ascontiguousarray(
            np.concatenate([a, b], axis=2).reshape(128, T * 2 * Kc)
        )

    in_maps = []
    for c in range(ncores):
        d = {}
        d["xT"] = np.roll(xT_pad, -c * NL, axis=1).astype(ml_dtypes.bfloat16)
        for t in TYPES:
            src_p, rel_p, ea_p, deg = pertype[t]
            src0 = (src_p - c * NL) % NPAD  # rotated layer-0 table layout
            rel_c = edge_layout(rel_p, c).astype(np.int32)
            d[f"ir0_{t}"] = interleave_ir(edge_layout(src0, c).astype(np.int32), rel_c)
            d[f"ir1_{t}"] = interleave_ir(edge_layout(remap_l1(src_p), c).astype(np.int32), rel_c)
            ea_c = ea_p[c * T : (c + 1) * T]  # [T, 4, K]
            d[f"ea_{t}"] = np.ascontiguousarray(
                ea_c.transpose(1, 0, 2).reshape(4, T * K)
            ).astype(ml_dtypes.bfloat16)
            d[f"deg_{t}"] = deg[c * NL : (c + 1) * NL].reshape(1, NL).astype(
                ml_dtypes.bfloat16
            )
        d["W_in"] = np.asarray(inputs["W_in"], np.float32).astype(ml_dtypes.bfloat16)
        d["b_in"] = np.asarray(inputs["b_in"], np.float32).reshape(H, 1)
        for t in TYPES:
            W1 = np.asarray(inputs[f"{t}_W1"], np.float32)
            b1 = np.asarray(inputs[f"{t}_b1"], np.float32)
            W2 = np.asarray(inputs[f"{t}_W2"], np.float32)
            b2 = np.asarray(inputs[f"{t}_b2"], np.float32)
            for l in range(L):
                d[f"w1i_{t}{l}"] = W1[l, :H].astype(ml_dtypes.bfloat16)
                d[f"w1j_{t}{l}"] = W1[l, H : 2 * H].astype(ml_dtypes.bfloat16)
                d[f"w1ab_{t}{l}"] = np.vstack([W1[l, 2 * H :], b1[l][None]]).astype(
                    ml_dtypes.bfloat16
                )
                d[f"w2_{t}{l}"] = W2[l].astype(ml_dtypes.bfloat16)
                d[f"b2r_{t}{l}"] = b2[l].reshape(1, H).astype(ml_dtypes.bfloat16)
        for l in range(L):
            d[f"combw_{l}"] = np.asarray(inputs["comb_W"], np.float32)[l].astype(
                ml_dtypes.bfloat16
            )
            d[f"combb_{l}"] = np.asarray(inputs["comb_b"], np.float32)[l].reshape(H, 1)
            d[f"lng_{l}"] = np.asarray(inputs["ln_g"], np.float32)[l].reshape(H, 1)
            d[f"lnb_{l}"] = np.asarray(inputs["ln_b"], np.float32)[l].reshape(H, 1)
        in_maps.append(d)
    return in_maps


def build(cfg, with_cc=True, repeat=1, ablate=()):
    ncores, T, Kc, K = cfg.n_cores, cfg.T, cfg.Kc, cfg.K
    NL, NPAD = cfg.n_local, cfg.NPAD
    nc = bass.Bass(trn_type="TRN2", num_devices=ncores)

    # ---- I/O ----
    xT = nc.dram_tensor("xT", [F_IN, NPAD], BF, kind="ExternalInput")
    din = {}
    for t in TYPES:
        din[f"ir0_{t}"] = nc.dram_tensor(f"ir0_{t}", [128, T * 2 * Kc], I32, kind="ExternalInput")
        din[f"ir1_{t}"] = nc.dram_tensor(f"ir1_{t}", [128, T * 2 * Kc], I32, kind="ExternalInput")
        din[f"ea_{t}"] = nc.dram_tensor(f"ea_{t}", [4, T * K], BF, kind="ExternalInput")
        din[f"deg_{t}"] = nc.dram_tensor(f"deg_{t}", [1, NL], BF, kind="ExternalInput")
        for l in range(L):
            din[f"w1i_{t}{l}"] = nc.dram_tensor(f"w1i_{t}{l}", [H, H], BF, kind="ExternalInput")
            din[f"w1j_{t}{l}"] = nc.dram_tensor(f"w1j_{t}{l}", [H, H], BF, kind="ExternalInput")
            din[f"w1ab_{t}{l}"] = nc.dram_tensor(f"w1ab_{t}{l}", [3, H], BF, kind="ExternalInput")
            din[f"w2_{t}{l}"] = nc.dram_tensor(f"w2_{t}{l}", [H, H], BF, kind="ExternalInput")
            din[f"b2r_{t}{l}"] = nc.dram_tensor(f"b2r_{t}{l}", [1, H], BF, kind="ExternalInput")
    WIN = nc.dram_tensor("W_in", [F_IN, H], BF, kind="ExternalInput")
    BIN = nc.dram_tensor("b_in", [H, 1], F32, kind="ExternalInput")
    for l in range(L):
        din[f"combw_{l}"] = nc.dram_tensor(f"combw_{l}", [H, H], BF, kind="ExternalInput")
        din[f"combb_{l}"] = nc.dram_tensor(f"combb_{l}", [H, 1], F32, kind="ExternalInput")
        din[f"lng_{l}"] = nc.dram_tensor(f"lng_{l}", [H, 1], F32, kind="ExternalInput")
        din[f"lnb_{l}"] = nc.dram_tensor(f"lnb_{l}", [H, 1], F32, kind="ExternalInput")
    out_rows = nc.dram_tensor("out", [NL, H], F32, kind="ExternalOutput")

    # ---- internal DRAM ----
    hfull0 = nc.dram_tensor("hfull0", [NPAD, H], BF)  # rotated per-core layout
    hfull1 = nc.dram_tensor("hfull1", [NPAD, H], BF, addr_space="Shared")  # global
    hsliceT = [nc.dram_tensor(f"hsliceT{l}", [H, NL], F32) for l in range(L)]
    hsliceTB = [nc.dram_tensor(f"hsliceTB{l}", [H, NL], BF) for l in range(L)]
    ccin = [
        nc.dram_tensor(f"ccin{i}", [cc[3], H], BF)
        for i, cc in enumerate(cfg.cc_chunks)
    ]
    hfull = [hfull0, hfull1]

    with tile.TileContext(nc) as tc:
        import contextlib

        ctx = contextlib.ExitStack()
        with ctx:
            cp = ctx.enter_context(tc.tile_pool(name="const", bufs=1))
            ident_bf = cp.tile([128, 128], BF)
            make_identity(nc, ident_bf[:])
            ident_f = cp.tile([128, 128], F32)
            make_identity(nc, ident_f[:])
            iota_i = cp.tile([128, 128], I32)
            nc.gpsimd.iota(iota_i[:], pattern=[[1, 128]], base=0, channel_multiplier=0)
            iota_part_f = cp.tile([128, 512], F32)
            nc.gpsimd.iota(iota_part_f[:], pattern=[[0, 512]], base=0, channel_multiplier=1,
                           allow_small_or_imprecise_dtypes=True)
            ones_col_bf = cp.tile([128, 1], BF)
            nc.gpsimd.memset(ones_col_bf[:], 1.0)
            ones_row_bf = cp.tile([1, 128], BF)
            nc.gpsimd.memset(ones_row_bf[:], 1.0)
            eps_col = cp.tile([128, 1], F32)
            nc.gpsimd.memset(eps_col[:], EPS)

            def load_const(name, shape, dtype):
                t_ = cp.tile(shape, dtype, tag=name, name=name)
                nc.sync.dma_start(out=t_[:], in_=din[name][:, :])
                return t_

            win_t = cp.tile([F_IN, H], BF)
            nc.sync.dma_start(out=win_t[:], in_=WIN[:, :])
            bin_t = cp.tile([H, 1], F32)
            nc.sync.dma_start(out=bin_t[:], in_=BIN[:, :])
            w1i = {}; w1j = {}; w1ab = {}; w2 = {}; b2r = {}
            for t in TYPES:
                for l in range(L):
                    w1i[t, l] = load_const(f"w1i_{t}{l}", [H, H], BF)
                    w1j[t, l] = load_const(f"w1j_{t}{l}", [H, H], BF)
                    w1ab[t, l] = load_const(f"w1ab_{t}{l}", [3, H], BF)
                    w2[t, l] = load_const(f"w2_{t}{l}", [H, H], BF)
                    b2r[t, l] = load_const(f"b2r_{t}{l}", [1, H], BF)
            combw = {}; combb = {}; lng = {}; lnb = {}
            for l in range(L):
                combw[l] = load_const(f"combw_{l}", [H, H], BF)
                combb[l] = load_const(f"combb_{l}", [H, 1], F32)
                lng[l] = load_const(f"lng_{l}", [H, 1], F32)
                lnb[l] = load_const(f"lnb_{l}", [H, 1], F32)

            # pools (PSUM banks: tr 2 + pre 2 + agg 2 + big 2 = 8)
            p_tr = ctx.enter_context(tc.tile_pool(name="ptr", bufs=2, space="PSUM"))
            p_pre = ctx.enter_context(tc.tile_pool(name="ppre", bufs=2, space="PSUM"))
            p_agg = ctx.enter_context(tc.tile_pool(name="pagg", bufs=1, space="PSUM"))
            p_big = ctx.enter_context(tc.tile_pool(name="pbig", bufs=2, space="PSUM"))
            sb = ctx.enter_context(tc.tile_pool(name="sb", bufs=3))
            sg = ctx.enter_context(tc.tile_pool(name="sg", bufs=24))
            srow = ctx.enter_context(tc.tile_pool(name="srow", bufs=3))
            sst = ctx.enter_context(tc.tile_pool(name="sst", bufs=2))
            snode = ctx.enter_context(tc.tile_pool(name="snode", bufs=2))
            saux = ctx.enter_context(tc.tile_pool(name="saux", bufs=2))

            # ================= h0 phase =================
            rep_range = range(repeat)
            n_h0 = NPAD // 512
            for rep in rep_range:
                own_chunks = (NL + 511) // 512
                for g in range(n_h0):
                    n0 = g * 512
                    xt = sb.tile([F_IN, 512], BF, tag="xt")
                    nc.sync.dma_start(out=xt[:], in_=xT[:, n0 : n0 + 512])
                    ph = p_big.tile([128, 512], F32, space="PSUM", tag="big")
                    nc.tensor.matmul(out=ph[:], lhsT=win_t[:], rhs=xt[:], start=True, stop=True)
                    h0b = sb.tile([128, 512], BF, tag="h0b")
                    nc.scalar.activation(out=h0b[:], in_=ph[:], func=AF.Relu, bias=bin_t[:, :1])
                    if g < own_chunks:
                        own_w = min(NL - n0, 512)
                        h0f = sb.tile([128, 512], F32, tag="h0f")
                        nc.scalar.activation(out=h0f[:, :own_w], in_=ph[:, :own_w], func=AF.Relu, bias=bin_t[:, :1])
                        nc.scalar.dma_start(out=hsliceT[0][:, n0 : n0 + own_w], in_=h0f[:, :own_w])
                        nc.scalar.dma_start(out=hsliceTB[0][:, n0 : n0 + own_w], in_=h0b[:, :own_w])
                    rw = srow.tile([128, 512], BF, tag="rw")
                    ptrw  = p_tr.tile([128, 512], BF, space="PSUM", tag="tr", name="ptrw ")
                    for j in range(4):
                        nc.tensor.transpose(out=ptrw [:, j * 128 : (j + 1) * 128], in_=h0b[:, j * 128 : (j + 1) * 128], identity=ident_bf[:])
                    nc.vector.tensor_copy(out=rw[:], in_=ptrw [:])
                    nc.scalar.dma_start(
                        out=hfull0[n0 : n0 + 512, :].rearrange("(j p) f -> p j f", j=4),
                        in_=rw[:].rearrange("p (j f) -> p j f", j=4),
                    )

                # ================= layers =================
                for l in range(L):
                    cc_done = [False] * len(cfg.cc_chunks)
                    ir_key = "ir0" if l == 0 else "ir1"
                    for gk, (k0, ntiles) in enumerate(cfg.groups):
                        W = ntiles * 128
                        n0g = k0 * 128
                        irg = {}
                        eag = {}
                        relrg = {}
                        for t in TYPES:
                            irg[t] = saux.tile([128, 4 * 2 * Kc], I32, tag=f"ir_{t}", name=f"ir_{t}")
                            nc.sync.dma_start(
                                out=irg[t][:, : ntiles * 2 * Kc],
                                in_=din[f"{ir_key}_{t}"][:, k0 * 2 * Kc : (k0 + ntiles) * 2 * Kc],
                            )
                            eag[t] = saux.tile([3, 4 * K], BF, tag=f"ea_{t}", name=f"ea_{t}")
                            nc.sync.dma_start(
                                out=eag[t][:, : ntiles * K],
                                in_=din[f"ea_{t}"][:3, k0 * K : (k0 + ntiles) * K],
                            )
                            relrg[t] = saux.tile([1, 4 * K], BF, tag=f"relr_{t}", name=f"relr_{t}")
                            nc.sync.dma_start(
                                out=relrg[t][:, : ntiles * K],
                                in_=din[f"ea_{t}"][3:4, k0 * K : (k0 + ntiles) * K],
                            )
                        degg = {}
                        for t in TYPES:
                            degg[t] = saux.tile([1, 512], BF, tag=f"deg_{t}", name=f"deg_{t}")
                            nc.sync.dma_start(out=degg[t][:, :W], in_=din[f"deg_{t}"][:, n0g : n0g + W])
                        hTg = saux.tile([128, 512], BF, tag="hTg")
                        nc.sync.dma_start(out=hTg[:, :W], in_=hsliceTB[l][:, n0g : n0g + W])
                        st = {t: sst.tile([128, 512], F32, tag=f"st_{t}", name=f"st_{t}") for t in TYPES}
                        for ki in range(ntiles):
                            k = k0 + ki
                            for t in TYPES:
                                phi = p_tr.tile([128, 128], F32, space="PSUM", tag="tr")
                                nc.tensor.matmul(
                                    out=phi[:], lhsT=hTg[:, ki * 128 : (ki + 1) * 128],
                                    rhs=w1i[t, l][:], start=True, stop=True,
                                )
                                hi = sb.tile([128, 128], BF, tag=f"hi_{t}", name=f"hi_{t}")
                                nc.scalar.copy(out=hi[:], in_=phi[:])
                                idxt = irg[t][:, ki * 2 * Kc : ki * 2 * Kc + Kc]
                                relt = irg[t][:, ki * 2 * Kc + Kc : (ki + 1) * 2 * Kc]
                                eat = eag[t][:3, ki * K : (ki + 1) * K]
                                relr = relrg[t][:1, ki * K : (ki + 1) * K]
                                if "slicegather" in ablate:
                                    gt1 = sg.tile([128, K], BF, tag="gbig", name="gbig", bufs=3)
                                    gts = [gt1[:, c * 128 : (c + 1) * 128] for c in range(Kc)]
                                    for c in range(Kc):
                                        nc.gpsimd.indirect_dma_start(
                                            out=gts[c],
                                            out_offset=None,
                                            in_=hfull[l][:, :],
                                            in_offset=bass.IndirectOffsetOnAxis(ap=idxt[:, c : c + 1], axis=0),
                                        )
                                else:
                                    gts = []
                                    for c in range(Kc):
                                        gtc = sg.tile([128, 128], BF, tag="g", name="g")
                                        nc.gpsimd.indirect_dma_start(
                                            out=gtc[:],
                                            out_offset=None,
                                            in_=hfull[l][:, :],
                                            in_offset=bass.IndirectOffsetOnAxis(ap=idxt[:, c : c + 1], axis=0),
                                        )
                                        gts.append(gtc)
                                A_all = sb.tile([128, K], BF, tag="A", name="A")
                                nc.vector.tensor_tensor(
                                    out=A_all[:].rearrange("p (c f) -> p c f", c=Kc),
                                    in0=relt[:, :, None].to_broadcast([128, Kc, 128]),
                                    in1=iota_i[:, None, :].to_broadcast([128, Kc, 128]),
                                    op=OP.is_equal,
                                )
                                msel = sb.tile([128, K], BF, tag="msel", name="msel")
                                for off in range(0, K, 512):
                                    w_ = min(512, K - off)
                                    pmsel = p_tr.tile([128, 512], F32, space="PSUM", tag="tr", name="pmsel")
                                    nc.tensor.matmul(
                                        out=pmsel[:, :w_], lhsT=ones_row_bf[:],
                                        rhs=relr[:, off : off + w_], start=True, stop=True,
                                    )
                                    nc.vector.tensor_tensor(
                                        out=msel[:, off : off + w_],
                                        in0=pmsel[:, :w_],
                                        in1=iota_part_f[:, :w_],
                                        op=OP.is_equal,
                                    )
                                agg = p_agg.tile([128, 128], F32, space="PSUM", tag=f"agg_{t}", name=f"agg_{t}")
                                for c in range(Kc):
                                    ptx = p_tr.tile([128, 128], BF, space="PSUM", tag="tr")
                                    nc.tensor.transpose(
                                        out=ptx[:], in_=gts[c] if "slicegather" in ablate else gts[c][:], identity=ident_bf[:]
                                    )
                                    xjt = sb.tile([128, 128], BF, tag="xjt")
                                    nc.scalar.copy(out=xjt[:], in_=ptx[:])
                                    pre = p_pre.tile([128, 128], F32, space="PSUM", tag="pre")
                                    nc.tensor.matmul(out=pre[:], lhsT=xjt[:], rhs=w1j[t, l][:], start=True, stop=False)
                                    nc.tensor.matmul(out=pre[:], lhsT=msel[:, c * 128 : (c + 1) * 128], rhs=hi[:], start=False, stop=False)
                                    nc.tensor.matmul(
                                        out=pre[:], lhsT=eat[:, c * 128 : (c + 1) * 128],
                                        rhs=w1ab[t, l][:], start=False, stop=True,
                                    )
                                    rt = sb.tile([128, 128], BF, tag="r")
                                    nc.scalar.activation(out=rt[:], in_=pre[:], func=AF.Relu)
                                    nc.tensor.matmul(
                                        out=agg[:], lhsT=rt[:], rhs=A_all[:, c * 128 : (c + 1) * 128],
                                        start=(c == 0), stop=(c == Kc - 1),
                                    )
                                nc.vector.tensor_copy(
                                    out=st[t][:, ki * 128 : (ki + 1) * 128], in_=agg[:]
                                )
                        # ---- node phase ----
                        stb = {}
                        for t in TYPES:
                            stb[t] = snode.tile([128, 512], BF, tag=f"stb_{t}", name=f"stb_{t}")
                            nc.vector.tensor_copy(out=stb[t][:, :W], in_=st[t][:, :W])
                        up = p_big.tile([128, 512], F32, space="PSUM", tag="big", name="up")
                        nc.tensor.matmul(out=up[:, :W], lhsT=w2["ally", l][:], rhs=stb["ally"][:, :W], start=True, stop=False)
                        nc.tensor.matmul(out=up[:, :W], lhsT=w2["enc", l][:], rhs=stb["enc"][:, :W], start=False, stop=False)
                        nc.tensor.matmul(out=up[:, :W], lhsT=b2r["ally", l][:], rhs=degg["ally"][:, :W], start=False, stop=False)
                        nc.tensor.matmul(out=up[:, :W], lhsT=b2r["enc", l][:], rhs=degg["enc"][:, :W], start=False, stop=True)
                        hold = snode.tile([128, 512], F32, tag="hold")
                        nc.sync.dma_start(out=hold[:, :W], in_=hsliceT[l][:, n0g : n0g + W])
                        s = snode.tile([128, 512], F32, tag="s")
                        nc.vector.tensor_tensor(out=s[:, :W], in0=up[:, :W], in1=hold[:, :W], op=OP.add)
                        sbf = snode.tile([128, 512], BF, tag="sbf")
                        nc.scalar.copy(out=sbf[:, :W], in_=s[:, :W])
                        prow = p_big.tile([128, 512], F32, space="PSUM", tag="big", name="prow")
                        nc.tensor.matmul(out=prow[0:1, :W], lhsT=ones_col_bf[:], rhs=sbf[:, :W], start=True, stop=True)
                        sq = snode.tile([128, 512], BF, tag="sq")
                        nc.scalar.square(out=sq[:, :W], in_=sbf[:, :W])
                        nc.tensor.matmul(out=prow[32:33, :W], lhsT=ones_col_bf[:], rhs=sq[:, :W], start=True, stop=True)
                        murow = snode.tile([1, 512], F32, tag="murow")
                        nc.vector.tensor_scalar_mul(out=murow[:, :W], in0=prow[0:1, :W], scalar1=1.0 / H)
                        e2row = snode.tile([1, 512], F32, tag="e2row")
                        nc.vector.tensor_scalar_mul(out=e2row[:, :W], in0=prow[32:33, :W], scalar1=1.0 / H)
                        musq = snode.tile([1, 512], F32, tag="musq")
                        nc.scalar.square(out=musq[:, :W], in_=murow[:, :W])
                        varr = snode.tile([1, 512], F32, tag="varr")
                        nc.vector.tensor_tensor(out=varr[:, :W], in0=e2row[:, :W], in1=musq[:, :W], op=OP.subtract)
                        sd = snode.tile([1, 512], F32, tag="sd")
                        nc.scalar.activation(out=sd[:, :W], in_=varr[:, :W], func=AF.Sqrt, bias=eps_col[:1, :1])
                        rstd = snode.tile([1, 512], F32, tag="rstd")
                        nc.vector.reciprocal(out=rstd[:, :W], in_=sd[:, :W])
                        mubf = snode.tile([1, 512], BF, tag="mubf")
                        nc.vector.tensor_copy(out=mubf[:, :W], in_=murow[:, :W])
                        rstdbf = snode.tile([1, 512], BF, tag="rstdbf")
                        nc.vector.tensor_copy(out=rstdbf[:, :W], in_=rstd[:, :W])
                        pmu = p_big.tile([128, 512], F32, space="PSUM", tag="big", name="pmu")
                        nc.tensor.matmul(out=pmu[:, :W], lhsT=ones_row_bf[:], rhs=mubf[:, :W], start=True, stop=True)
                        prs = p_big.tile([128, 512], F32, space="PSUM", tag="big", name="prs")
                        nc.tensor.matmul(out=prs[:, :W], lhsT=ones_row_bf[:], rhs=rstdbf[:, :W], start=True, stop=True)
                        t1 = snode.tile([128, 512], F32, tag="t1")
                        nc.vector.tensor_tensor(out=t1[:, :W], in0=s[:, :W], in1=pmu[:, :W], op=OP.subtract)
                        t2 = snode.tile([128, 512], F32, tag="t2")
                        nc.vector.tensor_tensor(out=t2[:, :W], in0=t1[:, :W], in1=prs[:, :W], op=OP.mult)
                        tt = snode.tile([128, 512], F32, tag="tt")
                        nc.vector.tensor_scalar(
                            out=tt[:, :W], in0=t2[:, :W],
                            scalar1=lng[l][:, :1], scalar2=lnb[l][:, :1],
                            op0=OP.mult, op1=OP.add,
                        )
                        ttbf = snode.tile([128, 512], BF, tag="ttbf")
                        nc.scalar.copy(out=ttbf[:, :W], in_=tt[:, :W])
                        pcm = p_big.tile([128, 512], F32, space="PSUM", tag="big", name="pcm")
                        nc.tensor.matmul(out=pcm[:, :W], lhsT=combw[l][:], rhs=ttbf[:, :W], start=True, stop=True)
                        s2 = snode.tile([128, 512], F32, tag="s2")
                        nc.scalar.activation(out=s2[:, :W], in_=pcm[:, :W], func=AF.Relu, bias=combb[l][:, :1])
                        hn = snode.tile([128, 512], F32, tag="hn")
                        nc.vector.tensor_tensor(out=hn[:, :W], in0=tt[:, :W], in1=s2[:, :W], op=OP.add)
                        if l == 0:
                            nc.scalar.dma_start(out=hsliceT[1][:, n0g : n0g + W], in_=hn[:, :W])
                            hnb = snode.tile([128, 512], BF, tag="hnb")
                            nc.scalar.copy(out=hnb[:, :W], in_=hn[:, :W])
                            nc.scalar.dma_start(out=hsliceTB[1][:, n0g : n0g + W], in_=hnb[:, :W])
                            rwn = srow.tile([128, 512], BF, tag="rwn")
                            ptrwn = p_tr.tile([128, 512], BF, space="PSUM", tag="tr", name="ptrwn")
                            for j in range(ntiles):
                                nc.tensor.transpose(out=ptrwn[:, j * 128 : (j + 1) * 128], in_=hnb[:, j * 128 : (j + 1) * 128], identity=ident_bf[:])
                            nc.vector.tensor_copy(out=rwn[:, :W], in_=ptrwn[:, :W])
                            ci = next(i for i, cc in enumerate(cfg.cc_chunks) if cc[0] <= gk < cc[1])
                            c0 = n0g - cfg.cc_chunks[ci][2]
                            nc.scalar.dma_start(
                                out=ccin[ci][c0 : c0 + W, :].rearrange("(j p) f -> p j f", j=ntiles),
                                in_=rwn[:, :W].rearrange("p (j f) -> p j f", j=ntiles),
                            )
                            # emit chunk-ci's AllGather LAGGED by 2 groups so the
                            # Pool engine reaches it after the node-phase writes
                            # have long completed (no Pool stall), still
                            # overlapping wire time with remaining compute
                            for cj, cc in enumerate(cfg.cc_chunks):
                                emit_at = min(cc[1] - 1 + 2, len(cfg.groups) - 1)
                                if with_cc and gk == emit_at and not cc_done[cj]:
                                    cc_done[cj] = True
                                    rows_c = cc[3]
                                    gb = sum(c2[3] for c2 in cfg.cc_chunks[:cj]) * ncores
                                    nc.gpsimd.collective_compute(
                                        "AllGather",
                                        OP.bypass,
                                        replica_groups=[list(range(ncores))],
                                        ins=[ccin[cj][:, :]],
                                        outs=[hfull1[gb : gb + ncores * rows_c, :]],
                                    )
                        else:
                            rwf = srow.tile([128, 512], F32, tag="rwf")
                            ptrwf = p_tr.tile([128, 512], F32, space="PSUM", tag="tr", name="ptrwf")
                            for j in range(ntiles):
                                nc.tensor.transpose(out=ptrwf[:, j * 128 : (j + 1) * 128], in_=hn[:, j * 128 : (j + 1) * 128], identity=ident_f[:])
                            nc.vector.tensor_copy(out=rwf[:, :W], in_=ptrwf[:, :W])
                            nc.scalar.dma_start(
                                out=out_rows[n0g : n0g + W, :].rearrange("(j p) f -> p j f", j=ntiles),
                                in_=rwf[:, :W].rearrange("p (j f) -> p j f", j=ntiles),
                            )
    return nc


N_CORES = 8
T_TILES = 98  # 128-node tiles per core -> NPAD = 100352 >= 100000


def kernel(**inputs):
    import numpy as np

    from concourse.bass_utils import run_bass_kernel_spmd

    np_inputs = {k: np.asarray(v) for k, v in inputs.items()}
    N = np_inputs["x"].shape[0]
    need = 1
    for t in TYPES:
        tid = np.asarray(np_inputs[f"{t}_ei"])[1].astype(np.int64) // 128
        cnt = np.bincount(tid, minlength=N_CORES * T_TILES)
        need = max(need, int(np.ceil(cnt.max() / 128)))
    cfg = Cfg(n_cores=N_CORES, T=T_TILES, Kc=need)
    assert cfg.NPAD >= N
    in_maps = host_prep(cfg, np_inputs)
    nc = build(cfg)
    res = run_bass_kernel_spmd(nc, in_maps, core_ids=list(range(N_CORES)))
    out = np.concatenate(
        [res.results[c]["out"] for c in range(N_CORES)], axis=0
    )[:N]
    return out.astype(np.float32)



# revision 16
# speedup vs baseline: 1.0436x; 1.0436x over previous
"""Trainium2 Bass kernel for the AgentEncoder GNN (nn_AgentEncoder_75840532512965).

Self-contained: kernel(**inputs) takes the FULL unsharded inputs (as in
reference.setup_inputs()), shards edges by target across 8 NeuronCores,
runs a Bass/Tile SPMD kernel via concourse.bass_utils.run_bass_kernel_spmd,
and returns the full [100000, 128] float32 output.

See the build() docstring for the kernel design.
"""
import concourse.tile as tile
from concourse.vector_clock import ScopedClock, VectorClock

"""Workaround for this walrus build: Drain/NoOp instructions (NO_STRUCT
codegen path) accept at most 2 sync waits. Tile's tail drain attaches one
wait per active logical processor, which can exceed that. Split the waits
one-per-NOP before the drain so the drain itself needs none."""

_orig_drain_and_barrier = tile.TileContext._drain_and_barrier


def _patched_drain_and_barrier(self, tick_clock, wait_clock):
    nc = self.nc
    gc = tick_clock.global_clock
    vec = list(gc)
    n = len(vec)
    for i, v in enumerate(vec):
        if v <= 0:
            continue
        partial = [0] * n
        partial[i] = v
        nopi = nc.sync.nop(nofuse=True, hint=f"tailwait{i}")
        wait_clock.add_sem_waits(
            nopi.ins, ScopedClock({None: VectorClock(partial)})
        )
    # replicate _orig_drain_and_barrier but with no waits on the drain
    # (SP executes it after the wait-nops above, same engine, in order)
    nc.sync.drain()
    nc.all_engine_barrier()
    popped = nc._tile_sem_poison_stack.pop()
    assert popped is self._sem_poison
    nc.clear_and_free_semaphores(list(self.sems.allocated().values()))
    nc.all_engine_barrier()


MAX_WAITS = 1


def _split_waits_json(raw: bytes) -> bytes:
    import json

    m = json.loads(raw)
    changed = False
    for f in m.get("functions", []):
        for bb in f.get("blocks", []):
            insts = bb.get("instructions", [])
            out = []
            for inst in insts:
                si = inst.get("sync_info")
                ow = (si or {}).get("on_wait") or []
                if len(ow) > MAX_WAITS:
                    extra = ow[:-MAX_WAITS]
                    keep = ow[-MAX_WAITS:]
                    si["on_wait"] = keep
                    for i in range(0, len(extra), MAX_WAITS):
                        out.append(
                            {
                                "debug": inst.get("debug", 0),
                                "engine": inst["engine"],
                                "ins": [],
                                "name": f"{inst['name']}_w{i}",
                                "opcode": "NoOp",
                                "outs": [],
                                "sync_info": {
                                    "on_update": [],
                                    "on_wait": extra[i : i + MAX_WAITS],
                                },
                                "text_hint": "waitsplit",
                            }
                        )
                    changed = True
                out.append(inst)
            bb["instructions"] = out
    if not changed:
        return raw
    return json.dumps(m).encode()


def _install_patches():
    import concourse.bass as bass

    tile.TileContext._drain_and_barrier = _patched_drain_and_barrier
    if not getattr(bass.Bass, "_waitsplit_patched", False):
        orig_to_json = bass.Bass.to_json_bytes

        def to_json_bytes(self, *a, **k):
            return _split_waits_json(orig_to_json(self, *a, **k))

        bass.Bass.to_json_bytes = to_json_bytes
        bass.Bass._waitsplit_patched = True


"""GNN message-passing kernel (AgentEncoder) for 8 TRN2 NeuronCores. v2.

Design:
- Nodes sharded by contiguous 128-node tiles: core c owns nodes
  [c*n_local, (c+1)*n_local), n_local = T*128. Edges sharded by TARGET.
- Within a core, edges sorted by target tile, padded per tile to K = Kc*128.
- Per tile+type: ONE batched indirect gather of all K source rows (bf16);
  one-hot selector matrices built on-device: A [e,n] via a single batched
  is_equal (3D broadcast APs), Msel [n,e] via ones-matmul broadcast of the
  rel-row + is_equal vs a partition-iota. Per 128-edge chunk: PE transpose
  of gathered rows, 3 accumulating matmuls for pre1, relu, scatter matmul.
- W2 applied per-node after aggregation (linearity), batched 4 tiles wide
  in feature-major layout; LN + comb-MLP fused there too.
- Layer 0 -> 1 h exchange: single AllGather (bf16).
- SPMD trick: layer-0 gather table is per-core ROTATED (own nodes first) so
  "own slice" addressing is uniform; indices are rotated on host. Layer-1
  table (AllGather output) is in global layout.
"""
import numpy as np
import ml_dtypes

import concourse.bass as bass
import concourse.mybir as mybir
import concourse.tile as tile
from concourse.masks import make_identity

_install_patches()

DT = mybir.dt
BF = DT.bfloat16
F32 = DT.float32
I32 = DT.int32
AF = mybir.ActivationFunctionType
OP = mybir.AluOpType

H = 128
F_IN = 39
L = 2
EPS = 1e-5
TYPES = ("ally", "enc")


class Cfg:
    def __init__(self, n_cores=8, T=98, Kc=7):
        self.n_cores = n_cores
        self.T = T            # 128-node tiles per core
        self.Kc = Kc          # 128-edge chunks per tile (padded)
        self.K = Kc * 128
        self.n_local = T * 128
        self.NPAD = n_cores * self.n_local
        self.groups = []
        k = 0
        while k < T:
            nt = min(4, T - k)
            self.groups.append((k, nt))
            k += nt
        # AllGather chunks: split groups into ~4 runs; per chunk (g_lo, g_hi, row_base, rows)
        ng = len(self.groups)
        if ng >= 8:
            # decreasing chunk sizes so the LAST AllGather (the exposed one)
            # is small; fractions of groups: 32%, 28%, 24%, 16%
            fr = [0.0, 0.36, 0.68, 0.92, 1.0]
            bounds = [round(f * ng) for f in fr]
        else:
            bounds = [round(i * ng / 4) for i in range(5)]
        self.cc_chunks = []
        for i in range(4):
            glo, ghi = bounds[i], bounds[i + 1]
            if glo == ghi:
                continue
            r0 = self.groups[glo][0] * 128
            r1 = (self.groups[ghi - 1][0] + self.groups[ghi - 1][1]) * 128
            self.cc_chunks.append((glo, ghi, r0, r1 - r0))


def host_prep(cfg, inputs):
    ncores, T, Kc, K = cfg.n_cores, cfg.T, cfg.Kc, cfg.K
    NL, NPAD = cfg.n_local, cfg.NPAD
    N = inputs["x"].shape[0]
    G = ncores * T

    x = np.asarray(inputs["x"], np.float32)
    xT_pad = np.zeros((F_IN, NPAD), np.float32)
    xT_pad[:, :N] = x.T

    pertype = {}
    for t in TYPES:
        ei = np.asarray(inputs[f"{t}_ei"])
        ea = np.asarray(inputs[f"{t}_ea"], np.float32)
        src, tgt = ei[0].astype(np.int64), ei[1].astype(np.int64)
        order = np.argsort(tgt, kind="stable")
        s_src, s_tgt, s_ea = src[order], tgt[order], ea[order]
        tile_id = s_tgt // 128
        counts = np.bincount(tile_id, minlength=G)
        need = int(np.ceil(counts.max() / 128))
        assert need <= Kc, f"Kc={Kc} too small, need {need}"
        offs = np.zeros(G + 1, np.int64)
        offs[1:] = np.cumsum(counts)
        col = np.arange(len(s_tgt)) - offs[tile_id]
        src_p = np.zeros((G, K), np.int64)
        rel_p = np.full((G, K), -1, np.int64)
        ea_p = np.zeros((G, 4, K), np.float32)
        src_p[tile_id, col] = s_src
        rel_p[tile_id, col] = s_tgt - tile_id * 128
        ea_p[tile_id, 0, col] = s_ea[:, 0]
        ea_p[tile_id, 1, col] = s_ea[:, 1]
        ea_p[tile_id, 2, col] = 1.0
        ea_p[:, 3, :] = rel_p  # rel row for Msel bcast (bf16-exact small ints)
        deg = np.bincount(tgt, minlength=NPAD).astype(np.float32)
        pertype[t] = (src_p, rel_p, ea_p, deg)

    # layer-1 table layout: chunk-major [sum_c 8*rows_c], position =
    # 8*base_c + rank*rows_c + (local - base_c)
    cbases = np.array([cc[2] for cc in cfg.cc_chunks], dtype=np.int64)
    crows = np.array([cc[3] for cc in cfg.cc_chunks], dtype=np.int64)
    gbase = np.concatenate([[0], np.cumsum(crows * ncores)])[:-1]

    def remap_l1(g):
        r = g // NL
        x = g % NL
        ci = np.searchsorted(cbases, x, side="right") - 1
        return gbase[ci] + r * crows[ci] + (x - cbases[ci])

    def edge_layout(arr_gk, c):
        # [T, K] for core c -> [128, T*Kc]: edge j of tile k -> (j%128, k*Kc + j//128)
        a = arr_gk[c * T : (c + 1) * T].reshape(T, Kc, 128)
        return np.ascontiguousarray(a.transpose(2, 0, 1).reshape(128, T * Kc))

    in_maps = []
    for c in range(ncores):
        d = {}
        d["xT"] = np.roll(xT_pad, -c * NL, axis=1).astype(ml_dtypes.bfloat16)
        for t in TYPES:
            src_p, rel_p, ea_p, deg = pertype[t]
            src0 = (src_p - c * NL) % NPAD  # rotated layer-0 table layout
            rel_c = edge_layout(rel_p, c).astype(np.int32)
            d[f"ix0_{t}"] = edge_layout(src0, c).astype(np.int32)
            d[f"ix1_{t}"] = edge_layout(remap_l1(src_p), c).astype(np.int32)
            d[f"relb_{t}"] = rel_c.astype(ml_dtypes.bfloat16)  # -1 pads exact
            ea_c = ea_p[c * T : (c + 1) * T]  # [T, 4, K]
            d[f"ea_{t}"] = np.ascontiguousarray(
                ea_c.transpose(1, 0, 2).reshape(4, T * K)
            ).astype(ml_dtypes.bfloat16)
            d[f"deg_{t}"] = deg[c * NL : (c + 1) * NL].reshape(1, NL).astype(
                ml_dtypes.bfloat16
            )
        d["W_in"] = np.asarray(inputs["W_in"], np.float32).astype(ml_dtypes.bfloat16)
        d["b_in"] = np.asarray(inputs["b_in"], np.float32).reshape(H, 1)
        for t in TYPES:
            W1 = np.asarray(inputs[f"{t}_W1"], np.float32)
            b1 = np.asarray(inputs[f"{t}_b1"], np.float32)
            W2 = np.asarray(inputs[f"{t}_W2"], np.float32)
            b2 = np.asarray(inputs[f"{t}_b2"], np.float32)
            for l in range(L):
                d[f"w1i_{t}{l}"] = W1[l, :H].astype(ml_dtypes.bfloat16)
                d[f"w1j_{t}{l}"] = W1[l, H : 2 * H].astype(ml_dtypes.bfloat16)
                d[f"w1ab_{t}{l}"] = np.vstack([W1[l, 2 * H :], b1[l][None]]).astype(
                    ml_dtypes.bfloat16
                )
                d[f"w2_{t}{l}"] = W2[l].astype(ml_dtypes.bfloat16)
                d[f"b2r_{t}{l}"] = b2[l].reshape(1, H).astype(ml_dtypes.bfloat16)
        for l in range(L):
            d[f"combw_{l}"] = np.asarray(inputs["comb_W"], np.float32)[l].astype(
                ml_dtypes.bfloat16
            )
            d[f"combb_{l}"] = np.asarray(inputs["comb_b"], np.float32)[l].reshape(H, 1)
            d[f"lng_{l}"] = np.asarray(inputs["ln_g"], np.float32)[l].reshape(H, 1)
            d[f"lnb_{l}"] = np.asarray(inputs["ln_b"], np.float32)[l].reshape(H, 1)
        in_maps.append(d)
    return in_maps


def build(cfg, with_cc=True, repeat=1, ablate=()):
    ncores, T, Kc, K = cfg.n_cores, cfg.T, cfg.Kc, cfg.K
    NL, NPAD = cfg.n_local, cfg.NPAD
    nc = bass.Bass(trn_type="TRN2", num_devices=ncores)

    # ---- I/O ----
    xT = nc.dram_tensor("xT", [F_IN, NPAD], BF, kind="ExternalInput")
    din = {}
    for t in TYPES:
        din[f"ix0_{t}"] = nc.dram_tensor(f"ix0_{t}", [128, T * Kc], I32, kind="ExternalInput")
        din[f"ix1_{t}"] = nc.dram_tensor(f"ix1_{t}", [128, T * Kc], I32, kind="ExternalInput")
        din[f"relb_{t}"] = nc.dram_tensor(f"relb_{t}", [128, T * Kc], BF, kind="ExternalInput")
        din[f"ea_{t}"] = nc.dram_tensor(f"ea_{t}", [4, T * K], BF, kind="ExternalInput")
        din[f"deg_{t}"] = nc.dram_tensor(f"deg_{t}", [1, NL], BF, kind="ExternalInput")
        for l in range(L):
            din[f"w1i_{t}{l}"] = nc.dram_tensor(f"w1i_{t}{l}", [H, H], BF, kind="ExternalInput")
            din[f"w1j_{t}{l}"] = nc.dram_tensor(f"w1j_{t}{l}", [H, H], BF, kind="ExternalInput")
            din[f"w1ab_{t}{l}"] = nc.dram_tensor(f"w1ab_{t}{l}", [3, H], BF, kind="ExternalInput")
            din[f"w2_{t}{l}"] = nc.dram_tensor(f"w2_{t}{l}", [H, H], BF, kind="ExternalInput")
            din[f"b2r_{t}{l}"] = nc.dram_tensor(f"b2r_{t}{l}", [1, H], BF, kind="ExternalInput")
    WIN = nc.dram_tensor("W_in", [F_IN, H], BF, kind="ExternalInput")
    BIN = nc.dram_tensor("b_in", [H, 1], F32, kind="ExternalInput")
    for l in range(L):
        din[f"combw_{l}"] = nc.dram_tensor(f"combw_{l}", [H, H], BF, kind="ExternalInput")
        din[f"combb_{l}"] = nc.dram_tensor(f"combb_{l}", [H, 1], F32, kind="ExternalInput")
        din[f"lng_{l}"] = nc.dram_tensor(f"lng_{l}", [H, 1], F32, kind="ExternalInput")
        din[f"lnb_{l}"] = nc.dram_tensor(f"lnb_{l}", [H, 1], F32, kind="ExternalInput")
    out_rows = nc.dram_tensor("out", [NL, H], F32, kind="ExternalOutput")

    # ---- internal DRAM ----
    hfull0 = nc.dram_tensor("hfull0", [NPAD, H], BF)  # rotated per-core layout
    hfull1 = nc.dram_tensor("hfull1", [NPAD, H], BF, addr_space="Shared")  # global
    hsliceT = [nc.dram_tensor(f"hsliceT{l}", [H, NL], F32) for l in range(L)]
    hsliceTB = [nc.dram_tensor(f"hsliceTB{l}", [H, NL], BF) for l in range(L)]
    ccin = [
        nc.dram_tensor(f"ccin{i}", [cc[3], H], BF)
        for i, cc in enumerate(cfg.cc_chunks)
    ]
    hfull = [hfull0, hfull1]

    with tile.TileContext(nc) as tc:
        import contextlib

        ctx = contextlib.ExitStack()
        with ctx:
            cp = ctx.enter_context(tc.tile_pool(name="const", bufs=1))
            ident_bf = cp.tile([128, 128], BF)
            make_identity(nc, ident_bf[:])
            ident_f = cp.tile([128, 128], F32)
            make_identity(nc, ident_f[:])
            iota_row_bf = cp.tile([128, 128], BF)
            nc.gpsimd.iota(iota_row_bf[:], pattern=[[1, 128]], base=0, channel_multiplier=0,
                           allow_small_or_imprecise_dtypes=True)
            iota_part_f = cp.tile([128, 512], F32)
            nc.gpsimd.iota(iota_part_f[:], pattern=[[0, 512]], base=0, channel_multiplier=1,
                           allow_small_or_imprecise_dtypes=True)
            ones_col_bf = cp.tile([128, 1], BF)
            nc.gpsimd.memset(ones_col_bf[:], 1.0)
            ones_row_bf = cp.tile([1, 128], BF)
            nc.gpsimd.memset(ones_row_bf[:], 1.0)
            eps_col = cp.tile([128, 1], F32)
            nc.gpsimd.memset(eps_col[:], EPS)

            def load_const(name, shape, dtype):
                t_ = cp.tile(shape, dtype, tag=name, name=name)
                nc.sync.dma_start(out=t_[:], in_=din[name][:, :])
                return t_

            win_t = cp.tile([F_IN, H], BF)
            nc.sync.dma_start(out=win_t[:], in_=WIN[:, :])
            bin_t = cp.tile([H, 1], F32)
            nc.sync.dma_start(out=bin_t[:], in_=BIN[:, :])
            w1i = {}; w1j = {}; w1ab = {}; w2 = {}; b2r = {}
            for t in TYPES:
                for l in range(L):
                    w1i[t, l] = load_const(f"w1i_{t}{l}", [H, H], BF)
                    w1j[t, l] = load_const(f"w1j_{t}{l}", [H, H], BF)
                    w1ab[t, l] = load_const(f"w1ab_{t}{l}", [3, H], BF)
                    w2[t, l] = load_const(f"w2_{t}{l}", [H, H], BF)
                    b2r[t, l] = load_const(f"b2r_{t}{l}", [1, H], BF)
            combw = {}; combb = {}; lng = {}; lnb = {}
            for l in range(L):
                combw[l] = load_const(f"combw_{l}", [H, H], BF)
                combb[l] = load_const(f"combb_{l}", [H, 1], F32)
                lng[l] = load_const(f"lng_{l}", [H, 1], F32)
                lnb[l] = load_const(f"lnb_{l}", [H, 1], F32)

            # pools (PSUM banks: tr 2 + pre 2 + agg 2 + big 2 = 8)
            p_tr = ctx.enter_context(tc.tile_pool(name="ptr", bufs=2, space="PSUM"))
            p_pre = ctx.enter_context(tc.tile_pool(name="ppre", bufs=2, space="PSUM"))
            p_agg = ctx.enter_context(tc.tile_pool(name="pagg", bufs=1, space="PSUM"))
            p_big = ctx.enter_context(tc.tile_pool(name="pbig", bufs=2, space="PSUM"))
            sb = ctx.enter_context(tc.tile_pool(name="sb", bufs=3))
            sg = ctx.enter_context(tc.tile_pool(name="sg", bufs=24))
            srow = ctx.enter_context(tc.tile_pool(name="srow", bufs=3))
            snode = ctx.enter_context(tc.tile_pool(name="snode", bufs=2))
            saux = ctx.enter_context(tc.tile_pool(name="saux", bufs=2))

            # ================= h0 phase =================
            rep_range = range(repeat)
            n_h0 = NPAD // 512
            for rep in rep_range:
                own_chunks = (NL + 511) // 512
                for g in range(n_h0):
                    n0 = g * 512
                    xt = sb.tile([F_IN, 512], BF, tag="xt")
                    nc.sync.dma_start(out=xt[:], in_=xT[:, n0 : n0 + 512])
                    ph = p_big.tile([128, 512], F32, space="PSUM", tag="big")
                    nc.tensor.matmul(out=ph[:], lhsT=win_t[:], rhs=xt[:], start=True, stop=True)
                    h0b = sb.tile([128, 512], BF, tag="h0b")
                    nc.scalar.activation(out=h0b[:], in_=ph[:], func=AF.Relu, bias=bin_t[:, :1])
                    if g < own_chunks:
                        own_w = min(NL - n0, 512)
                        h0f = sb.tile([128, 512], F32, tag="h0f")
                        nc.scalar.activation(out=h0f[:, :own_w], in_=ph[:, :own_w], func=AF.Relu, bias=bin_t[:, :1])
                        nc.scalar.dma_start(out=hsliceT[0][:, n0 : n0 + own_w], in_=h0f[:, :own_w])
                        nc.scalar.dma_start(out=hsliceTB[0][:, n0 : n0 + own_w], in_=h0b[:, :own_w])
                    rw = srow.tile([128, 512], BF, tag="rw")
                    ptrw  = p_tr.tile([128, 512], BF, space="PSUM", tag="tr", name="ptrw ")
                    for j in range(4):
                        nc.tensor.transpose(out=ptrw [:, j * 128 : (j + 1) * 128], in_=h0b[:, j * 128 : (j + 1) * 128], identity=ident_bf[:])
                    nc.vector.tensor_copy(out=rw[:], in_=ptrw [:])
                    nc.scalar.dma_start(
                        out=hfull0[n0 : n0 + 512, :].rearrange("(j p) f -> p j f", j=4),
                        in_=rw[:].rearrange("p (j f) -> p j f", j=4),
                    )

                # ================= layers =================
                for l in range(L):
                    cc_done = [False] * len(cfg.cc_chunks)
                    ir_key = "ix0" if l == 0 else "ix1"
                    for gk, (k0, ntiles) in enumerate(cfg.groups):
                        W = ntiles * 128
                        n0g = k0 * 128
                        irg = {}
                        relg = {}
                        eag = {}
                        relrg = {}
                        for t in TYPES:
                            irg[t] = saux.tile([128, 4 * Kc], I32, tag=f"ix_{t}", name=f"ix_{t}")
                            nc.sync.dma_start(
                                out=irg[t][:, : ntiles * Kc],
                                in_=din[f"{ir_key}_{t}"][:, k0 * Kc : (k0 + ntiles) * Kc],
                            )
                            relg[t] = saux.tile([128, 4 * Kc], BF, tag=f"relg_{t}", name=f"relg_{t}")
                            nc.sync.dma_start(
                                out=relg[t][:, : ntiles * Kc],
                                in_=din[f"relb_{t}"][:, k0 * Kc : (k0 + ntiles) * Kc],
                            )
                            eag[t] = saux.tile([3, 4 * K], BF, tag=f"ea_{t}", name=f"ea_{t}")
                            nc.sync.dma_start(
                                out=eag[t][:, : ntiles * K],
                                in_=din[f"ea_{t}"][:3, k0 * K : (k0 + ntiles) * K],
                            )
                            relrg[t] = saux.tile([1, 4 * K], BF, tag=f"relr_{t}", name=f"relr_{t}")
                            nc.sync.dma_start(
                                out=relrg[t][:, : ntiles * K],
                                in_=din[f"ea_{t}"][3:4, k0 * K : (k0 + ntiles) * K],
                            )
                        degg = {}
                        for t in TYPES:
                            degg[t] = saux.tile([1, 512], BF, tag=f"deg_{t}", name=f"deg_{t}")
                            nc.sync.dma_start(out=degg[t][:, :W], in_=din[f"deg_{t}"][:, n0g : n0g + W])
                        hTg = saux.tile([128, 512], BF, tag="hTg")
                        nc.sync.dma_start(out=hTg[:, :W], in_=hsliceTB[l][:, n0g : n0g + W])
                        stb = {t: snode.tile([128, 512], BF, tag=f"stb_{t}", name=f"stb_{t}") for t in TYPES}
                        # hi for all tiles of the group, one batched copy per type
                        hi4 = {}
                        for t in TYPES:
                            phi4 = p_tr.tile([128, 512], F32, space="PSUM", tag="tr", name="phi4")
                            for ki in range(ntiles):
                                nc.tensor.matmul(
                                    out=phi4[:, ki * 128 : (ki + 1) * 128],
                                    lhsT=hTg[:, ki * 128 : (ki + 1) * 128],
                                    rhs=w1i[t, l][:], start=True, stop=True,
                                )
                            hi4[t] = sb.tile([128, 512], BF, tag=f"hi_{t}", name=f"hi_{t}")
                            nc.scalar.copy(out=hi4[t][:, :W], in_=phi4[:, :W])
                        for ki in range(ntiles):
                            k = k0 + ki
                            for t in TYPES:
                                hit = hi4[t][:, ki * 128 : (ki + 1) * 128]
                                idxt = irg[t][:, ki * Kc : (ki + 1) * Kc]
                                relt = relg[t][:, ki * Kc : (ki + 1) * Kc]
                                eat = eag[t][:3, ki * K : (ki + 1) * K]
                                relr = relrg[t][:1, ki * K : (ki + 1) * K]
                                # per-chunk indirect gathers into one big tile
                                # (HW DGE pairs exactly one index per partition)
                                gt1 = sg.tile([128, K], BF, tag="gbig", name="gbig", bufs=4)
                                if "nogather" not in ablate:
                                    for c in range(Kc):
                                        nc.gpsimd.indirect_dma_start(
                                            out=gt1[:, c * 128 : (c + 1) * 128],
                                            out_offset=None,
                                            in_=hfull[l][:, :],
                                            in_offset=bass.IndirectOffsetOnAxis(ap=idxt[:, c : c + 1], axis=0),
                                        )
                                A_all = sb.tile([128, K], BF, tag="A", name="A")
                                nc.vector.tensor_tensor(
                                    out=A_all[:].rearrange("p (c f) -> p c f", c=Kc),
                                    in0=relt[:, :, None].to_broadcast([128, Kc, 128]),
                                    in1=iota_row_bf[:, None, :].to_broadcast([128, Kc, 128]),
                                    op=OP.is_equal,
                                )
                                msel = sb.tile([128, K], BF, tag="msel", name="msel")
                                for off in range(0, K, 512):
                                    w_ = min(512, K - off)
                                    pmsel = p_tr.tile([128, 512], F32, space="PSUM", tag="tr", name="pmsel")
                                    nc.tensor.matmul(
                                        out=pmsel[:, :w_], lhsT=ones_row_bf[:],
                                        rhs=relr[:, off : off + w_], start=True, stop=True,
                                    )
                                    nc.vector.tensor_tensor(
                                        out=msel[:, off : off + w_],
                                        in0=pmsel[:, :w_],
                                        in1=iota_part_f[:, :w_],
                                        op=OP.is_equal,
                                    )
                                agg = p_agg.tile([128, 128], F32, space="PSUM", tag=f"agg_{t}", name=f"agg_{t}")
                                for cq in range(0, Kc, 4):
                                    nb = min(4, Kc - cq)
                                    wq = nb * 128
                                    ptx4 = p_tr.tile([128, 512], BF, space="PSUM", tag="tr", name="ptx4")
                                    for j in range(nb):
                                        nc.tensor.transpose(
                                            out=ptx4[:, j * 128 : (j + 1) * 128],
                                            in_=gt1[:, (cq + j) * 128 : (cq + j + 1) * 128],
                                            identity=ident_bf[:],
                                        )
                                    xjt4 = sb.tile([128, 512], BF, tag="xjt")
                                    nc.scalar.copy(out=xjt4[:, :wq], in_=ptx4[:, :wq])
                                    pre4 = p_pre.tile([128, 512], F32, space="PSUM", tag="pre")
                                    for j in range(nb):
                                        c = cq + j
                                        sl = slice(j * 128, (j + 1) * 128)
                                        nc.tensor.matmul(out=pre4[:, sl], lhsT=xjt4[:, sl], rhs=w1j[t, l][:], start=True, stop=False)
                                        nc.tensor.matmul(out=pre4[:, sl], lhsT=msel[:, c * 128 : (c + 1) * 128], rhs=hit, start=False, stop=False)
                                        nc.tensor.matmul(
                                            out=pre4[:, sl], lhsT=eat[:, c * 128 : (c + 1) * 128],
                                            rhs=w1ab[t, l][:], start=False, stop=True,
                                        )
                                    rt4 = sb.tile([128, 512], BF, tag="r")
                                    nc.scalar.activation(out=rt4[:, :wq], in_=pre4[:, :wq], func=AF.Relu)
                                    for j in range(nb):
                                        c = cq + j
                                        nc.tensor.matmul(
                                            out=agg[:], lhsT=rt4[:, j * 128 : (j + 1) * 128],
                                            rhs=A_all[:, c * 128 : (c + 1) * 128],
                                            start=(c == 0), stop=(c == Kc - 1),
                                        )
                                nc.vector.tensor_copy(
                                    out=stb[t][:, ki * 128 : (ki + 1) * 128], in_=agg[:]
                                )
                        # ---- node phase ----
                        up = p_big.tile([128, 512], F32, space="PSUM", tag="big", name="up")
                        nc.tensor.matmul(out=up[:, :W], lhsT=w2["ally", l][:], rhs=stb["ally"][:, :W], start=True, stop=False)
                        nc.tensor.matmul(out=up[:, :W], lhsT=w2["enc", l][:], rhs=stb["enc"][:, :W], start=False, stop=False)
                        nc.tensor.matmul(out=up[:, :W], lhsT=b2r["ally", l][:], rhs=degg["ally"][:, :W], start=False, stop=False)
                        nc.tensor.matmul(out=up[:, :W], lhsT=b2r["enc", l][:], rhs=degg["enc"][:, :W], start=False, stop=True)
                        hold = snode.tile([128, 512], F32, tag="hold")
                        nc.sync.dma_start(out=hold[:, :W], in_=hsliceT[l][:, n0g : n0g + W])
                        s = snode.tile([128, 512], F32, tag="s")
                        nc.vector.tensor_tensor(out=s[:, :W], in0=up[:, :W], in1=hold[:, :W], op=OP.add)
                        sbf = snode.tile([128, 512], BF, tag="sbf")
                        nc.scalar.copy(out=sbf[:, :W], in_=s[:, :W])
                        prow = p_big.tile([128, 512], F32, space="PSUM", tag="big", name="prow")
                        nc.tensor.matmul(out=prow[0:1, :W], lhsT=ones_col_bf[:], rhs=sbf[:, :W], start=True, stop=True)
                        sq = snode.tile([128, 512], BF, tag="sq")
                        nc.scalar.square(out=sq[:, :W], in_=sbf[:, :W])
                        nc.tensor.matmul(out=prow[32:33, :W], lhsT=ones_col_bf[:], rhs=sq[:, :W], start=True, stop=True)
                        murow = snode.tile([1, 512], F32, tag="murow")
                        nc.vector.tensor_scalar_mul(out=murow[:, :W], in0=prow[0:1, :W], scalar1=1.0 / H)
                        e2row = snode.tile([1, 512], F32, tag="e2row")
                        nc.vector.tensor_scalar_mul(out=e2row[:, :W], in0=prow[32:33, :W], scalar1=1.0 / H)
                        musq = snode.tile([1, 512], F32, tag="musq")
                        nc.scalar.square(out=musq[:, :W], in_=murow[:, :W])
                        varr = snode.tile([1, 512], F32, tag="varr")
                        nc.vector.tensor_tensor(out=varr[:, :W], in0=e2row[:, :W], in1=musq[:, :W], op=OP.subtract)
                        sd = snode.tile([1, 512], F32, tag="sd")
                        nc.scalar.activation(out=sd[:, :W], in_=varr[:, :W], func=AF.Sqrt, bias=eps_col[:1, :1])
                        rstd = snode.tile([1, 512], F32, tag="rstd")
                        nc.vector.reciprocal(out=rstd[:, :W], in_=sd[:, :W])
                        mubf = snode.tile([1, 512], BF, tag="mubf")
                        nc.vector.tensor_copy(out=mubf[:, :W], in_=murow[:, :W])
                        rstdbf = snode.tile([1, 512], BF, tag="rstdbf")
                        nc.vector.tensor_copy(out=rstdbf[:, :W], in_=rstd[:, :W])
                        pmu = p_big.tile([128, 512], F32, space="PSUM", tag="big", name="pmu")
                        nc.tensor.matmul(out=pmu[:, :W], lhsT=ones_row_bf[:], rhs=mubf[:, :W], start=True, stop=True)
                        prs = p_big.tile([128, 512], F32, space="PSUM", tag="big", name="prs")
                        nc.tensor.matmul(out=prs[:, :W], lhsT=ones_row_bf[:], rhs=rstdbf[:, :W], start=True, stop=True)
                        t1 = snode.tile([128, 512], F32, tag="t1")
                        nc.vector.tensor_tensor(out=t1[:, :W], in0=s[:, :W], in1=pmu[:, :W], op=OP.subtract)
                        t2 = snode.tile([128, 512], F32, tag="t2")
                        nc.vector.tensor_tensor(out=t2[:, :W], in0=t1[:, :W], in1=prs[:, :W], op=OP.mult)
                        tt = snode.tile([128, 512], F32, tag="tt")
                        nc.vector.tensor_scalar(
                            out=tt[:, :W], in0=t2[:, :W],
                            scalar1=lng[l][:, :1], scalar2=lnb[l][:, :1],
                            op0=OP.mult, op1=OP.add,
                        )
                        ttbf = snode.tile([128, 512], BF, tag="ttbf")
                        nc.scalar.copy(out=ttbf[:, :W], in_=tt[:, :W])
                        pcm = p_big.tile([128, 512], F32, space="PSUM", tag="big", name="pcm")
                        nc.tensor.matmul(out=pcm[:, :W], lhsT=combw[l][:], rhs=ttbf[:, :W], start=True, stop=True)
                        s2 = snode.tile([128, 512], F32, tag="s2")
                        nc.scalar.activation(out=s2[:, :W], in_=pcm[:, :W], func=AF.Relu, bias=combb[l][:, :1])
                        hn = snode.tile([128, 512], F32, tag="hn")
                        nc.vector.tensor_tensor(out=hn[:, :W], in0=tt[:, :W], in1=s2[:, :W], op=OP.add)
                        if l == 0:
                            nc.scalar.dma_start(out=hsliceT[1][:, n0g : n0g + W], in_=hn[:, :W])
                            hnb = snode.tile([128, 512], BF, tag="hnb")
                            nc.scalar.copy(out=hnb[:, :W], in_=hn[:, :W])
                            nc.scalar.dma_start(out=hsliceTB[1][:, n0g : n0g + W], in_=hnb[:, :W])
                            rwn = srow.tile([128, 512], BF, tag="rwn")
                            ptrwn = p_tr.tile([128, 512], BF, space="PSUM", tag="tr", name="ptrwn")
                            for j in range(ntiles):
                                nc.tensor.transpose(out=ptrwn[:, j * 128 : (j + 1) * 128], in_=hnb[:, j * 128 : (j + 1) * 128], identity=ident_bf[:])
                            nc.vector.tensor_copy(out=rwn[:, :W], in_=ptrwn[:, :W])
                            ci = next(i for i, cc in enumerate(cfg.cc_chunks) if cc[0] <= gk < cc[1])
                            c0 = n0g - cfg.cc_chunks[ci][2]
                            nc.scalar.dma_start(
                                out=ccin[ci][c0 : c0 + W, :].rearrange("(j p) f -> p j f", j=ntiles),
                                in_=rwn[:, :W].rearrange("p (j f) -> p j f", j=ntiles),
                            )
                            # emit chunk-ci's AllGather LAGGED by 2 groups so the
                            # Pool engine reaches it after the node-phase writes
                            # have long completed (no Pool stall), still
                            # overlapping wire time with remaining compute
                            for cj, cc in enumerate(cfg.cc_chunks):
                                emit_at = min(cc[1] - 1 + 2, len(cfg.groups) - 1)
                                if with_cc and gk == emit_at and not cc_done[cj]:
                                    cc_done[cj] = True
                                    rows_c = cc[3]
                                    gb = sum(c2[3] for c2 in cfg.cc_chunks[:cj]) * ncores
                                    nc.gpsimd.collective_compute(
                                        "AllGather",
                                        OP.bypass,
                                        replica_groups=[list(range(ncores))],
                                        ins=[ccin[cj][:, :]],
                                        outs=[hfull1[gb : gb + ncores * rows_c, :]],
                                    )
                        else:
                            rwf = srow.tile([128, 512], F32, tag="rwf")
                            ptrwf = p_tr.tile([128, 512], F32, space="PSUM", tag="tr", name="ptrwf")
                            for j in range(ntiles):
                                nc.tensor.transpose(out=ptrwf[:, j * 128 : (j + 1) * 128], in_=hn[:, j * 128 : (j + 1) * 128], identity=ident_f[:])
                            nc.vector.tensor_copy(out=rwf[:, :W], in_=ptrwf[:, :W])
                            nc.scalar.dma_start(
                                out=out_rows[n0g : n0g + W, :].rearrange("(j p) f -> p j f", j=ntiles),
                                in_=rwf[:, :W].rearrange("p (j f) -> p j f", j=ntiles),
                            )
    return nc


N_CORES = 8
T_TILES = 98  # 128-node tiles per core -> NPAD = 100352 >= 100000


def kernel(**inputs):
    import numpy as np

    from concourse.bass_utils import run_bass_kernel_spmd

    np_inputs = {k: np.asarray(v) for k, v in inputs.items()}
    N = np_inputs["x"].shape[0]
    need = 1
    for t in TYPES:
        tid = np.asarray(np_inputs[f"{t}_ei"])[1].astype(np.int64) // 128
        cnt = np.bincount(tid, minlength=N_CORES * T_TILES)
        need = max(need, int(np.ceil(cnt.max() / 128)))
    cfg = Cfg(n_cores=N_CORES, T=T_TILES, Kc=need)
    assert cfg.NPAD >= N
    in_maps = host_prep(cfg, np_inputs)
    nc = build(cfg)
    res = run_bass_kernel_spmd(nc, in_maps, core_ids=list(range(N_CORES)))
    out = np.concatenate(
        [res.results[c]["out"] for c in range(N_CORES)], axis=0
    )[:N]
    return out.astype(np.float32)

